# revision 29
# baseline (speedup 1.0000x reference)
"""Trainium2 Bass kernel for a dense transformer block (pre-LN, MHA + MLP).

Sharding: 8 cores; core c handles batch b = c // 4, query block qb = c % 4
(512 tokens). Each core recomputes K/V for its batch's full 2048-token
sequence (zero cross-core communication). The host rotates each core's
sequence so device chunk 0 == the core's query block; attention/softmax are
order-invariant over kv chunks, so the device program is identical (SPMD).

Precision plan (validated vs fp64 reference in numpy):
- activations/weights bf16 (DVE 2x, half DMA/SBUF, PE rate = fp32r)
- Q/K/V projections in fp8 e4m3 with DoubleRow (2 contraction tiles per
  pass); LN1 output is stored fp8 in the natural [P, ct, tok] layout that
  DoubleRow's [Ki, 2, N] moving-operand AP wants
- PSUM, residual stream (x2), LN stats in fp32
- LayerNorm gains are folded into the following matmul weights host-side;
  LN biases fold into per-output bias vectors (exactly), V's bias folds into
  the proj bias via softmax sum-to-one
- rstd = exp(-0.5*ln(var+eps)) keeps ACT inside the natural_log_exp table
  set: no ACT table swaps until the single gelu swap in the MLP
- softmax runs without max-subtraction (scores are small), denominators via
  a ones-row appended to V inside the AV matmul, accumulated in the same
  [65, TQ] PSUM->SBUF add as the AV outputs
"""
import os

import numpy as np
import ml_dtypes

import concourse.bass as bass
import concourse.mybir as mybir
import concourse.tile as tile
from concourse import bacc
from concourse.bass_utils import run_bass_kernel_spmd

P = 128
C = 1024
NCT = C // P          # 8 feature tiles
NG = C // 256         # 4 DoubleRow contraction groups
TKV = 2048            # kv tokens per core (sequence length)
TQ = 512              # query tokens per core
HID = 4096
NHT = HID // P        # 32 hidden tiles
H = 16
HD = 64
NHP = H // 2          # 8 head pairs
CHUNK = 512           # kv tokens processed per pipeline chunk
NCHUNK = TKV // CHUNK # 4
NJCL = CHUNK // P     # 4 j-subchunks of 128 per chunk
EPS = 1e-5
SCALE = HD ** -0.5

f32 = mybir.dt.float32
f32r = mybir.dt.float32r
bf16 = mybir.dt.bfloat16
fp8 = mybir.dt.float8e4
Act = mybir.ActivationFunctionType
Alu = mybir.AluOpType
DR = mybir.MatmulPerfMode.DoubleRow


def build_program(sim_standin=False, debug=False):
    # CoreSim lacks Gelu; Tanh has identical ACT cost, so the sim variant
    # swaps it in for modeled-time runs (numerics then checked vs a matching
    # numpy reference).
    gelu_fn = Act.Tanh if sim_standin else Act.Gelu
    nc = bacc.Bacc()

    xkvT = nc.dram_tensor("xkvT", [C, TKV], bf16, kind="ExternalInput")
    wq8 = nc.dram_tensor("wq8", [P, NG * 2 * C], fp8, kind="ExternalInput")
    wk8 = nc.dram_tensor("wk8", [P, NG * 2 * C], fp8, kind="ExternalInput")
    wv8 = nc.dram_tensor("wv8", [P, NG * 2 * C], fp8, kind="ExternalInput")
    wpS = nc.dram_tensor("wpS", [P, NHP * C], bf16, kind="ExternalInput")
    w1S = nc.dram_tensor("w1S", [P, NCT * HID], bf16, kind="ExternalInput")
    w2S = nc.dram_tensor("w2S", [P, NHT * C], bf16, kind="ExternalInput")
    bqD = nc.dram_tensor("bqD", [P, NHP], f32, kind="ExternalInput")
    bkD = nc.dram_tensor("bkD", [P, NHP], f32, kind="ExternalInput")
    bpD = nc.dram_tensor("bpD", [P, NCT], f32, kind="ExternalInput")
    b1D = nc.dram_tensor("b1D", [P, NHT], f32, kind="ExternalInput")
    b2D = nc.dram_tensor("b2D", [P, NCT], f32, kind="ExternalInput")
    outT = nc.dram_tensor("outT", [C, TQ], f32, kind="ExternalOutput")
    if debug:
        dbg = {
            "d_ln0": nc.dram_tensor("d_ln0", [P, NCT, CHUNK], fp8,
                                    kind="ExternalOutput"),
            "d_qT": nc.dram_tensor("d_qT", [P, NHP, TQ], bf16,
                                   kind="ExternalOutput"),
            "d_kT": nc.dram_tensor("d_kT", [P, NHP, CHUNK], bf16,
                                   kind="ExternalOutput"),
            "d_vc": nc.dram_tensor("d_vc", [P, NJCL, H, HD + 1], bf16,
                                   kind="ExternalOutput"),
            "d_att": nc.dram_tensor("d_att", [HD + 1, H, TQ], bf16,
                                    kind="ExternalOutput"),
            "d_anb": nc.dram_tensor("d_anb", [P, NHP, TQ], bf16,
                                    kind="ExternalOutput"),
            "d_x2": nc.dram_tensor("d_x2", [P, NCT, TQ], f32r,
                                   kind="ExternalOutput"),
            "d_ln2": nc.dram_tensor("d_ln2", [P, NCT, TQ], bf16,
                                    kind="ExternalOutput"),
        }

    with tile.TileContext(nc) as tc:
      with (
          tc.tile_pool(name="const", bufs=1) as const,
          tc.tile_pool(name="pw8", bufs=1) as pw8,
          tc.tile_pool(name="px2", bufs=1) as px2,
          tc.tile_pool(name="pxq", bufs=1) as pxq,
          tc.tile_pool(name="pq", bufs=1) as pq,
      ):
        # ---- program-start DMAs: x chunk 0 first (critical path) ----
        xq_x = pxq.tile([P, NCT, CHUNK], bf16, name="xq_x")
        nc.sync.dma_start(
            xq_x[:], xkvT[:, 0:CHUNK].rearrange("(ct p) f -> p ct f", p=P))

        wq8_t = pw8.tile([P, NG, 2, C], fp8, name="wq8_t")
        nc.sync.dma_start(
            wq8_t[:], wq8[:].rearrange("p (g l c) -> p g l c", g=NG, l=2))
        wk8_t = pw8.tile([P, NG, 2, C], fp8, name="wk8_t")
        nc.sync.dma_start(
            wk8_t[:], wk8[:].rearrange("p (g l c) -> p g l c", g=NG, l=2))
        wv8_t = pw8.tile([P, NG, 2, C], fp8, name="wv8_t")
        nc.sync.dma_start(
            wv8_t[:], wv8[:].rearrange("p (g l c) -> p g l c", g=NG, l=2))

        ones_bf = const.tile([P, HD], bf16)
        nc.vector.memset(ones_bf[:], 1.0)
        ones_stat = const.tile([P, 1], bf16)
        nc.vector.tensor_copy(ones_stat[:], ones_bf[:, 0:1])
        ones_f32 = const.tile([P, 1], f32r)
        nc.vector.memset(ones_f32[:], 1.0)
        eps_t = const.tile([P, 1], f32)
        nc.vector.memset(eps_t[:], EPS)

        bq_t = const.tile([P, NHP], f32, name="bq_t")
        nc.sync.dma_start(bq_t[:], bqD[:])
        bk_t = const.tile([P, NHP], f32, name="bk_t")
        nc.sync.dma_start(bk_t[:], bkD[:])
        bp_t = const.tile([P, NCT], f32, name="bp_t")
        nc.sync.dma_start(bp_t[:], bpD[:])
        b1_t = const.tile([P, NHT], f32, name="b1_t")
        nc.sync.dma_start(b1_t[:], b1D[:])
        b2_t = const.tile([P, NCT], f32, name="b2_t")
        nc.sync.dma_start(b2_t[:], b2D[:])

        x2T = px2.tile([P, NCT, TQ], f32r, name="x2T")
        qT = pq.tile([P, NHP, TQ], bf16, name="qT")
        attn_bf = pq.tile([P, NHP, TQ], bf16, name="attn_bf")
        ln2T = pq.tile([P, NCT, TQ], bf16, name="ln2T")

        with tc.tile_pool(name="patt", bufs=1) as patt:
            # attnT2[p, h, t]: rows 0:64 AV accum for head h, row 64 its
            # softmax denominator (matches the [65, TQ] AV PSUM tile)
            attnT2 = patt.tile([P, H, TQ], bf16, name="attnT2")
            kT_c = patt.tile([P, NHP, CHUNK], bf16, name="kT_c")
            v_c = patt.tile([P, NJCL, H, HD + 1], bf16, name="v_c")
            nc.vector.tensor_copy(
                v_c[:, :, :, HD],
                ones_bf[:].rearrange("p (a b) -> p a b", a=NJCL))

            with (
                tc.tile_pool(name="px", bufs=2) as px,
                tc.tile_pool(name="pln", bufs=2) as pln,
                tc.tile_pool(name="pe", bufs=2) as pe,
                tc.tile_pool(name="psb", bufs=2) as psb,
                tc.tile_pool(name="pst", bufs=2) as pst,
                tc.tile_pool(name="pb_ps", bufs=1, space="PSUM") as pb_ps,
                tc.tile_pool(name="pb_psav", bufs=1, space="PSUM") as pb_psav,
            ):
                def emit_ln(x_t, F, out_dt=fp8):
                    """LN stats+apply over feature dim; returns [P,NCT,F]."""
                    ps_stat = pb_ps.tile([1, 2 * F], f32, tag="sc_ps",
                                         bufs=2, name="ps_stat")
                    for ct in range(NCT):
                        x_ct = x_t[:, ct, :]
                        sq = psb.tile([P, F], bf16, tag="ln_sq", bufs=3)
                        nc.vector.tensor_mul(sq[:], x_ct, x_ct)
                        nc.tensor.matmul(ps_stat[:, 0:F], ones_stat[:], x_ct,
                                         start=(ct == 0), stop=(ct == NCT - 1))
                        nc.tensor.matmul(ps_stat[:, F:2 * F], ones_stat[:],
                                         sq[:],
                                         start=(ct == 0), stop=(ct == NCT - 1))
                    nmu = pst.tile([1, F], f32, tag="ln_nmu")
                    ex2 = pst.tile([1, F], f32, tag="ln_ex2")
                    nc.vector.tensor_scalar_mul(nmu[:], ps_stat[:, 0:F],
                                                -1.0 / C)
                    nc.vector.tensor_scalar_mul(ex2[:], ps_stat[:, F:2 * F],
                                                1.0 / C)
                    mu2 = pst.tile([1, F], f32, tag="ln_mu2")
                    nc.vector.tensor_mul(mu2[:], nmu[:], nmu[:])
                    nc.vector.tensor_sub(ex2[:], ex2[:], mu2[:])
                    # rstd = exp(-0.5 * ln(var + eps)): stays in the
                    # natural_log_exp ACT table set (no table swap)
                    lnv = pst.tile([1, F], f32, tag="ln_lnv")
                    nc.scalar.activation(lnv[:], ex2[:], Act.Ln,
                                         bias=eps_t[0:1, :])
                    c1 = pst.tile([1, F], bf16, tag="ln_c1")
                    nc.scalar.activation(c1[:], lnv[:], Act.Exp, scale=-0.5)
                    c0 = pst.tile([1, F], bf16, tag="ln_c0")
                    nc.vector.tensor_mul(c0[:], nmu[:], c1[:])
                    c1_b = psb.tile([P, F], bf16, tag="ln_c1b")
                    c0_b = psb.tile([P, F], bf16, tag="ln_c0b")
                    nc.gpsimd.partition_broadcast(c1_b[:], c1[:])
                    nc.gpsimd.partition_broadcast(c0_b[:], c0[:])
                    out = pln.tile([P, NCT, F], out_dt, tag="lnkv")
                    for ct in range(NCT):
                        t1 = psb.tile([P, F], bf16, tag="ln_t1", bufs=3)
                        nc.vector.tensor_mul(t1[:], x_t[:, ct, :], c1_b[:])
                        nc.vector.tensor_add(out[:, ct, :], t1[:], c0_b[:])
                    return out

                def emit_ln_chunk(ch):
                    j0 = ch * CHUNK
                    xkv_t = px.tile([P, NCT, CHUNK], bf16, tag="xkv")
                    nc.sync.dma_start(
                        xkv_t[:],
                        xkvT[:, j0:j0 + CHUNK].rearrange(
                            "(ct p) f -> p ct f", p=P))
                    return emit_ln(xkv_t, CHUNK)

                def emit_k(lnc, hp):
                    ps = pb_ps.tile([P, CHUNK], f32, tag="kv_ps", bufs=2,
                                    name="k_ps")
                    for g in range(NG):
                        nc.tensor.matmul(
                            ps[:], wk8_t[:, g, :, hp * P:(hp + 1) * P],
                            lnc[:, 2 * g:2 * g + 2, :],
                            start=(g == 0), stop=(g == NG - 1), perf_mode=DR)
                    nc.vector.tensor_scalar_add(kT_c[:, hp, :], ps[:],
                                                bk_t[:, hp:hp + 1])

                def emit_q(lnc, hp):
                    ps = pb_ps.tile([P, CHUNK], f32, tag="kv_ps", bufs=2,
                                    name="q_ps")
                    for g in range(NG):
                        nc.tensor.matmul(
                            ps[:], wq8_t[:, g, :, hp * P:(hp + 1) * P],
                            lnc[:, 2 * g:2 * g + 2, :],
                            start=(g == 0), stop=(g == NG - 1), perf_mode=DR)
                    nc.vector.tensor_scalar_add(qT[:, hp, :], ps[:],
                                                bq_t[:, hp:hp + 1])

                def emit_v(lnc, jl):
                    for half in range(2):
                        ps = pb_ps.tile([P, CHUNK], f32, tag="kv_ps", bufs=2,
                                        name="v_ps")
                        for g in range(NG):
                            nc.tensor.matmul(
                                ps[:],
                                lnc[:, 2 * g:2 * g + 2, jl * P:(jl + 1) * P],
                                wv8_t[:, g, :, half * 512:(half + 1) * 512],
                                start=(g == 0), stop=(g == NG - 1),
                                perf_mode=DR)
                        nc.vector.tensor_copy(
                            v_c[:, jl, half * 8:(half + 1) * 8, 0:HD],
                            ps[:].rearrange("p (h d) -> p h d", d=HD))

                def emit_attn(ch, hp):
                    ps_av0 = pb_psav.tile([HD + 1, TQ], f32, tag="av0",
                                          name="ps_av0")
                    ps_av1 = pb_psav.tile([HD + 1, TQ], f32, tag="av1",
                                          name="ps_av1")
                    for jl in range(NJCL):
                        ps_sc = pb_ps.tile([P, 2 * TQ], f32, tag="sc_ps",
                                           bufs=2, name="ps_sc")
                        nc.tensor.matmul(
                            ps_sc[:, 0:TQ],
                            kT_c[0:HD, hp, jl * P:(jl + 1) * P],
                            qT[0:HD, hp, :], start=True, stop=True)
                        nc.tensor.matmul(
                            ps_sc[:, TQ:2 * TQ],
                            kT_c[HD:P, hp, jl * P:(jl + 1) * P],
                            qT[HD:P, hp, :], start=True, stop=True)
                        e_sb = pe.tile([P, 2 * TQ], bf16, tag="e")
                        nc.scalar.activation(e_sb[:], ps_sc[:], Act.Exp,
                                             scale=SCALE)
                        nc.tensor.matmul(
                            ps_av0[:], v_c[:, jl, 2 * hp, :],
                            e_sb[:, 0:TQ],
                            start=(jl == 0), stop=(jl == NJCL - 1))
                        nc.tensor.matmul(
                            ps_av1[:], v_c[:, jl, 2 * hp + 1, :],
                            e_sb[:, TQ:2 * TQ],
                            start=(jl == 0), stop=(jl == NJCL - 1))
                    for i, ps_av in ((0, ps_av0), (1, ps_av1)):
                        h = 2 * hp + i
                        dst = attnT2[0:HD + 1, h, :]
                        if ch == 0:
                            nc.vector.tensor_copy(dst, ps_av[:])
                        else:
                            nc.vector.tensor_add(dst, dst, ps_av[:])

                # ---- Phase A: LN chunk 0, Q, K0, V0 ----
                _sidA = nc.enter_named_scope("A_lnq", False)[0]
                ln_t = {0: emit_ln(xq_x, CHUNK)}
                for hp in range(NHP):
                    emit_q(ln_t[0], hp)
                for hp in range(NHP):
                    emit_k(ln_t[0], hp)
                for jl in range(NJCL):
                    emit_v(ln_t[0], jl)
                ln_t[1] = emit_ln_chunk(1)
                nc.leave_named_scope("A_lnq", _sidA, False)

                # ---- Phase B: attention chunks with next-chunk K/V woven in
                for ch in range(NCHUNK):
                    _sidC = nc.enter_named_scope(f"B_ch{ch}", False)[0]
                    for hp in range(NHP):
                        emit_attn(ch, hp)
                        if ch + 1 < NCHUNK:
                            emit_k(ln_t[ch + 1], hp)
                    if ch + 1 < NCHUNK:
                        # only after every head consumed this chunk's v_c
                        for jl in range(NJCL):
                            emit_v(ln_t[ch + 1], jl)
                    if ch + 2 < NCHUNK:
                        ln_t[ch + 2] = emit_ln_chunk(ch + 2)
                    nc.leave_named_scope(f"B_ch{ch}", _sidC, False)

                # ---- normalize: attn_bf[.., hp, :] = attn / den ----
                _sidN = nc.enter_named_scope("B_norm", False)[0]
                den16 = psb.tile([H, TQ], bf16, tag="den16", bufs=1)
                nc.sync.dma_start(den16[:], attnT2[HD:HD + 1, :, :])
                rcp16 = psb.tile([H, TQ], bf16, tag="rcp16", bufs=1)
                with nc.allow_low_precision(reason="softmax rcp in bf16"):
                    nc.vector.reciprocal(rcp16[:], den16[:])
                rcp_st = psb.tile([1, H, TQ], bf16, tag="rcp_st", bufs=1)
                nc.sync.dma_start(rcp_st[:], rcp16[:])
                for h in range(H):
                    rcp_b = psb.tile([HD, TQ], bf16, tag="rcp_b", bufs=3)
                    nc.gpsimd.partition_broadcast(rcp_b[:],
                                                  rcp_st[0:1, h, :])
                    nc.vector.tensor_mul(
                        attn_bf[(h % 2) * HD:(h % 2 + 1) * HD, h // 2, :],
                        attnT2[0:HD, h, :], rcp_b[:])
                nc.leave_named_scope("B_norm", _sidN, False)

                if debug:
                    nc.sync.dma_start(dbg["d_ln0"][:], ln_t[0][:])
                    nc.sync.dma_start(dbg["d_qT"][:], qT[:])
                    nc.sync.dma_start(dbg["d_kT"][:], kT_c[:])
                    nc.sync.dma_start(dbg["d_vc"][:], v_c[:])
                    nc.sync.dma_start(dbg["d_att"][:], attnT2[0:HD + 1, :, :])
                    nc.sync.dma_start(dbg["d_anb"][:], attn_bf[:])

                # ---- Phase C: proj + residual; LN2 stats interleaved ----
                _sidPC = nc.enter_named_scope("C_proj", False)[0]
                ps_stat2 = pb_ps.tile([1, 2 * TQ], f32, tag="sc_ps", bufs=2,
                                      name="ps_stat2")
                for ct in range(NCT):
                    wp_t = psb.tile([P, NHP, P], bf16, tag="wp", bufs=2)
                    nc.sync.dma_start(
                        wp_t[:],
                        wpS[:].rearrange("p (hp c) -> p hp c", hp=NHP)[
                            :, :, ct * P:(ct + 1) * P])
                    ps = pb_ps.tile([P, TQ], f32, tag="kv_ps", bufs=2,
                                    name="proj_ps")
                    for hp in range(NHP):
                        nc.tensor.matmul(
                            ps[:], wp_t[:, hp, :], attn_bf[:, hp, :],
                            start=(hp == 0), stop=(hp == NHP - 1))
                    o = x2T[:, ct, :]
                    nc.vector.scalar_tensor_tensor(
                        o, ps[:], bp_t[:, ct:ct + 1], xq_x[:, ct, :],
                        op0=Alu.add, op1=Alu.add)
                    # LN2 stats as soon as this ct exists
                    sq = psb.tile([P, TQ], f32r, tag="sq2", bufs=3)
                    nc.vector.tensor_mul(sq[:], o, o)
                    nc.tensor.matmul(ps_stat2[:, 0:TQ], ones_f32[:], o,
                                     start=(ct == 0), stop=(ct == NCT - 1))
                    nc.tensor.matmul(ps_stat2[:, TQ:2 * TQ], ones_f32[:],
                                     sq[:],
                                     start=(ct == 0), stop=(ct == NCT - 1))
                nc.leave_named_scope("C_proj", _sidPC, False)

                # ---- Phase D prologue: finish LN2 ----
                _sidD1 = nc.enter_named_scope("D_fc1", False)[0]
                nmu = pst.tile([1, TQ], f32, tag="ln_nmu")
                ex2 = pst.tile([1, TQ], f32, tag="ln_ex2")
                nc.vector.tensor_scalar_mul(nmu[:], ps_stat2[:, 0:TQ],
                                            -1.0 / C)
                nc.vector.tensor_scalar_mul(ex2[:], ps_stat2[:, TQ:2 * TQ],
                                            1.0 / C)
                mu2 = pst.tile([1, TQ], f32, tag="ln_mu2")
                nc.vector.tensor_mul(mu2[:], nmu[:], nmu[:])
                nc.vector.tensor_sub(ex2[:], ex2[:], mu2[:])
                lnv = pst.tile([1, TQ], f32, tag="ln_lnv")
                nc.scalar.activation(lnv[:], ex2[:], Act.Ln,
                                         bias=eps_t[0:1, :])
                c1 = pst.tile([1, TQ], bf16, tag="ln_c1")
                nc.scalar.activation(c1[:], lnv[:], Act.Exp, scale=-0.5)
                c0 = pst.tile([1, TQ], bf16, tag="ln_c0")
                nc.vector.tensor_mul(c0[:], nmu[:], c1[:])
                c1_b = psb.tile([P, TQ], bf16, tag="ln_c1b")
                c0_b = psb.tile([P, TQ], bf16, tag="ln_c0b")
                nc.gpsimd.partition_broadcast(c1_b[:], c1[:])
                nc.gpsimd.partition_broadcast(c0_b[:], c0[:])
                for ct in range(NCT):
                    t1 = psb.tile([P, TQ], bf16, tag="ln_t1", bufs=3)
                    nc.vector.tensor_mul(t1[:], x2T[:, ct, :], c1_b[:])
                    nc.vector.tensor_add(ln2T[:, ct, :], t1[:], c0_b[:])
                if debug:
                    nc.sync.dma_start(dbg["d_x2"][:], x2T[:])
                    nc.sync.dma_start(dbg["d_ln2"][:], ln2T[:])

        # ---- Phase D: fc1+gelu, fc2 + residual ----
        with (
            tc.tile_pool(name="pd_sb", bufs=3) as pd_sb,
            tc.tile_pool(name="pd_g", bufs=1) as pd_g,
            tc.tile_pool(name="pd_w", bufs=2) as pd_w,
            tc.tile_pool(name="pd_ps", bufs=2, space="PSUM") as pd_ps,
            tc.tile_pool(name="pd_ps2", bufs=1, space="PSUM") as pd_ps2,
        ):
            g1T = pd_g.tile([P, NHT, TQ], bf16)
            w1v = w1S[:].rearrange("p (ct h) -> p ct h", ct=NCT)
            for htg in range(NHT // 4):
                w1_t = pd_w.tile([P, NCT, 512], bf16, tag="w1")
                nc.sync.dma_start(
                    w1_t[:], w1v[:, :, htg * 512:(htg + 1) * 512])
                for hl in range(4):
                    ht = htg * 4 + hl
                    ps = pd_ps.tile([P, TQ], f32, tag="fc1_ps")
                    for ct in range(NCT):
                        nc.tensor.matmul(
                            ps[:], w1_t[:, ct, hl * P:(hl + 1) * P],
                            ln2T[:, ct, :],
                            start=(ct == 0), stop=(ct == NCT - 1))
                    nc.scalar.activation(g1T[:, ht, :], ps[:], gelu_fn,
                                         bias=b1_t[:, ht:ht + 1])
            nc.leave_named_scope("D_fc1", _sidD1, False)

            _sidD2 = nc.enter_named_scope("D_fc2", False)[0]
            w2v = w2S[:].rearrange("p (ht c) -> p ht c", ht=NHT)
            for ctg in range(2):
                ps_out = [pd_ps2.tile([P, TQ], f32, tag=f"fc2_{i}",
                                      name=f"fc2_ps_{i}")
                          for i in range(4)]
                for htg4 in range(NHT // 4):
                    w2_t = pd_w.tile([P, 4, 512], bf16, tag="w2", bufs=3)
                    nc.sync.dma_start(
                        w2_t[:],
                        w2v[:, htg4 * 4:(htg4 + 1) * 4,
                            ctg * 512:(ctg + 1) * 512])
                    for hl in range(4):
                        ht = htg4 * 4 + hl
                        for cl in range(4):
                            nc.tensor.matmul(
                                ps_out[cl][:],
                                w2_t[:, hl, cl * P:(cl + 1) * P],
                                g1T[:, ht, :],
                                start=(ht == 0), stop=(ht == NHT - 1))
                for cl in range(4):
                    ct = ctg * 4 + cl
                    o = pd_sb.tile([P, TQ], f32, tag="out_t")
                    nc.vector.scalar_tensor_tensor(
                        o[:], ps_out[cl][:], b2_t[:, ct:ct + 1],
                        x2T[:, ct, :], op0=Alu.add, op1=Alu.add)
                    nc.sync.dma_start(outT[ct * P:(ct + 1) * P, :], o[:])
            nc.leave_named_scope("D_fc2", _sidD2, False)

    nc.finalize()
    return nc


def _dr8(w):
    """fp8 DoubleRow layout: out[p, g, l, m] = w[m, (2g+l)*128+p]."""
    a = np.ascontiguousarray(
        w.T.reshape(NG, 2, P, w.shape[0]).transpose(2, 0, 1, 3))
    return a.astype(ml_dtypes.float8_e4m3).reshape(P, -1)


def _col128(v):
    """[n*128] bias vector -> [128, n] with col j = v[j*128:(j+1)*128]."""
    return np.ascontiguousarray(v.reshape(-1, P).T.astype(np.float32))


def _host_weights(inputs):
    f32a = lambda k: np.asarray(inputs[k], dtype=np.float32)
    w_qkv, w_proj = f32a("w_qkv"), f32a("w_proj")
    w_fc1, w_fc2 = f32a("w_fc1"), f32a("w_fc2")
    ln1_g, ln1_b = f32a("ln1_g"), f32a("ln1_b")
    ln2_g, ln2_b = f32a("ln2_g"), f32a("ln2_b")

    wg = w_qkv * ln1_g[None, :]
    bqkv = w_qkv @ ln1_b
    bq, bk, bv = bqkv[0:C], bqkv[C:2 * C], bqkv[2 * C:]
    bp = f32a("b_proj") + w_proj @ bv
    w1g = w_fc1 * ln2_g[None, :]
    b1 = f32a("b_fc1") + w_fc1 @ ln2_b

    def sbuf_lhs(w):  # [p, r, o] = w[o, r*128+p], bf16, flattened
        r = w.shape[1] // P
        a = w.T.reshape(r, P, w.shape[0]).transpose(1, 0, 2)
        return np.ascontiguousarray(a).astype(
            ml_dtypes.bfloat16).reshape(P, -1)

    return {
        "wq8": _dr8(wg[0:C]),
        "wk8": _dr8(wg[C:2 * C]),
        "wv8": _dr8(wg[2 * C:]),
        "wpS": sbuf_lhs(w_proj),
        "w1S": sbuf_lhs(w1g),
        "w2S": sbuf_lhs(w_fc2),
        "bqD": _col128(bq),
        "bkD": _col128(bk),
        "bpD": _col128(bp),
        "b1D": _col128(b1),
        "b2D": _col128(f32a("b_fc2")),
    }


def _rot_xT(x_b, qb):
    """[N, C] batch slice -> bf16 [C, N] with token block qb rotated first."""
    xT = np.asarray(x_b, dtype=np.float32).T
    r = np.concatenate([xT[:, qb * TQ:], xT[:, :qb * TQ]], axis=1)
    return np.ascontiguousarray(r).astype(ml_dtypes.bfloat16)


def sim_feed(inputs, core=0):
    feed = dict(_host_weights(inputs))
    b, qb = core // 4, core % 4
    feed["xkvT"] = _rot_xT(np.asarray(inputs["x"])[b], qb)
    return feed


_program = None


def _get_program():
    global _program
    if _program is None:
        _program = build_program()
    return _program


def kernel(**inputs):
    x = np.asarray(inputs["x"], dtype=np.float32)
    B, N, _ = x.shape  # [2, 2048, 1024]

    shared = _host_weights(inputs)
    in_maps = []
    for core in range(8):
        b, qb = core // 4, core % 4
        m = dict(shared)
        m["xkvT"] = _rot_xT(x[b], qb)
        in_maps.append(m)

    nc = _get_program()
    trace = bool(os.environ.get("BASS_TRACE"))
    kw = {}
    if trace:
        os.makedirs("/tmp/trace_out", exist_ok=True)
        kw = dict(tmpdir="/tmp/trace_out", trace=True)
    res = run_bass_kernel_spmd(nc, in_maps, list(range(8)), **kw)
    if trace:
        if res.exec_time_ns is not None:
            print(f"HW exec time: {res.exec_time_ns} ns")
        print(f"mean exec: {res.mean_exec_time_ns} ns, "
              f"max core: {res.max_exec_time_core_id}")
        if res.per_core_scope_times:
            for scope, per_core in sorted(res.per_core_scope_times.items()):
                print(f"scope {scope}: {per_core}")
        print(f"profile_json: {res.profile_json}")

    out = np.empty((B, N, C), dtype=np.float32)
    for core in range(8):
        b, qb = core // 4, core % 4
        out[b, qb * TQ:(qb + 1) * TQ, :] = res.results[core]["outT"].T
    return out
